# revision 10
# baseline (speedup 1.0000x reference)
"""GQA attention (B=2,S=2048,E=2048,H=16,KV=4,D=128, RoPE, causal) on 8 trn2 cores.

Sharding: core c = (b = c//4, kv = c%4). Tensor-parallel over kv-head groups
(Wq cols / Wk,Wv cols / Wo rows) x data-parallel over batch. Each core computes
a full [S, E] partial output (its head group's contribution) in bf16; host sums
the 4 partials per batch element in f32.

All inputs are cast to bf16 on the HOST (no cast DMAs on device). Layout:
  qT/kT [d, s] = Wq_chunk.T @ xT   (PSUM accum over e-chunks)
  rot(q) via a single 128x128 permutation matmul (PermT.T @ qraw), then
  rope = qraw*cos + rot*sin on DVE (one extra matmul instead of a second
  full projection).
  v     [s, d] = xT_chunk.T @ Wv
  scoresT [sk, sq] = kT_chunk.T @ qT_block -> exp (no max-subtraction; scores
     are ~N(0,0.8)). Causal handling: strips above the block-diagonal are
     skipped; diagonal strips compute only the valid column suffix, and the
     single 128x128 boundary block is masked with a triangular 0/1 multiply.
  rowsum: DVE accumulates exp tiles (bf16) -> one ones-matmul per (h, block)
     -> 1/x via ACT exp(-log(x)) -> broadcast with a K=1 matmul.
  outT [d, sq] += v_chunk.T @ expT  (PSUM accum over sk-chunks)
  y [sq, e] += outT_norm_chunk.T @ Wo_head  (accum over 4 heads), bf16 out.
"""
import sys
sys.path.insert(0, "/opt/trn_rl_repo")
import numpy as np
import ml_dtypes

BF = ml_dtypes.bfloat16

B, S, E = 2, 2048, 2048
H, KV, D = 16, 4, 128
G = H // KV          # 4 q heads per kv head / core
THETA = 10000.0
P = 128
NE = E // P          # 16 e-chunks
NB = 4               # s-blocks per core loop
BS = S // NB         # 512
NSC = S // P         # 16 s-chunks

_CACHE = {}


def _build():
    if "nc" in _CACHE:
        return _CACHE["nc"]
    import concourse.bass as bass
    import concourse.tile as tile
    from concourse import mybir, bacc, bass_isa

    f32 = mybir.dt.float32
    bf16 = mybir.dt.bfloat16
    EXP = mybir.ActivationFunctionType.Exp
    LN = mybir.ActivationFunctionType.Ln
    SCALE = 1.0 / np.sqrt(D)

    nc = bacc.Bacc("TRN2", target_bir_lowering=False, debug=False)
    xT_d = nc.declare_dram_parameter("xT", [E, S], bf16, isOutput=False)
    wq_d = nc.declare_dram_parameter("wq", [E, G * D], bf16, isOutput=False)
    wk_d = nc.declare_dram_parameter("wk", [E, D], bf16, isOutput=False)
    wv_d = nc.declare_dram_parameter("wv", [E, D], bf16, isOutput=False)
    wo_d = nc.declare_dram_parameter("wo", [G * D, E], bf16, isOutput=False)
    cos_d = nc.declare_dram_parameter("cosT", [P, S], bf16, isOutput=False)
    sin_d = nc.declare_dram_parameter("sinT", [P, S], f32, isOutput=False)
    tri_d = nc.declare_dram_parameter("tri", [P, P], bf16, isOutput=False)
    perm_d = nc.declare_dram_parameter("perm", [P, P], bf16, isOutput=False)
    y_d = nc.declare_dram_parameter("y", [S, E], bf16, isOutput=True)

    with tile.TileContext(nc) as tc, \
         nc.allow_low_precision(reason="bf16 matmul pipeline"):
        import contextlib
        with contextlib.ExitStack() as ctx:
            cst = ctx.enter_context(tc.tile_pool(name="cst", bufs=1))
            wqp = ctx.enter_context(tc.tile_pool(name="wqp", bufs=16))
            wkvp = ctx.enter_context(tc.tile_pool(name="wkvp", bufs=32))
            wop = ctx.enter_context(tc.tile_pool(name="wop", bufs=4))
            xtp = ctx.enter_context(tc.tile_pool(name="xtp", bufs=48))
            kvp = ctx.enter_context(tc.tile_pool(name="kvp", bufs=1))
            vp = ctx.enter_context(tc.tile_pool(name="vp", bufs=16))
            qtp = ctx.enter_context(tc.tile_pool(name="qtp", bufs=8))
            rawp = ctx.enter_context(tc.tile_pool(name="rawp", bufs=4))
            rtp = ctx.enter_context(tc.tile_pool(name="rtp", bufs=6))
            exp_p = ctx.enter_context(tc.tile_pool(name="exp", bufs=6))
            esp = ctx.enter_context(tc.tile_pool(name="esp", bufs=2))
            recp = ctx.enter_context(tc.tile_pool(name="recp", bufs=4))
            otp = ctx.enter_context(tc.tile_pool(name="otp", bufs=8))
            ybp = ctx.enter_context(tc.tile_pool(name="ybp", bufs=3))
            psS = ctx.enter_context(tc.tile_pool(name="psS", bufs=2, space="PSUM"))
            psP = ctx.enter_context(tc.tile_pool(name="psP", bufs=2, space="PSUM"))
            psO = ctx.enter_context(tc.tile_pool(name="psO", bufs=2, space="PSUM"))
            psY = ctx.enter_context(tc.tile_pool(name="psY", bufs=2, space="PSUM"))

            # ---- constants / weights (resident) ----
            cos_sb = cst.tile([P, S], bf16, tag="cos")
            sin_sb = cst.tile([P, S], f32, tag="sin")
            tri_sb = cst.tile([P, P], bf16, tag="tri")
            perm_sb = cst.tile([P, P], bf16, tag="perm")
            nc.sync.dma_start(cos_sb[:], cos_d[:])
            nc.sync.dma_start(sin_sb[:], sin_d[:])
            nc.gpsimd.dma_start(tri_sb[:], tri_d[:])
            nc.gpsimd.dma_start(perm_sb[:], perm_d[:])
            ones_col = cst.tile([P, 1], bf16, tag="onc")
            nc.vector.memset(ones_col[:], 1.0)
            ones_row = cst.tile([1, P], bf16, tag="onr")
            nc.vector.memset(ones_row[:], 1.0)

            wk_sb, wv_sb = [], []
            for e in range(NE):
                t = wkvp.tile([P, D], bf16, tag="wk")
                nc.gpsimd.dma_start(t[:], wk_d[e * P:(e + 1) * P, :])
                wk_sb.append(t)
                t = wkvp.tile([P, D], bf16, tag="wv")
                nc.gpsimd.dma_start(t[:], wv_d[e * P:(e + 1) * P, :])
                wv_sb.append(t)
            wq_sb = []
            for e in range(NE):
                t = wqp.tile([P, G * D], bf16, tag="wq")
                nc.gpsimd.dma_start(t[:], wq_d[e * P:(e + 1) * P, :])
                wq_sb.append(t)
            wo_sb = []
            for h in range(G):
                t = wop.tile([P, E], bf16, tag="wo")
                nc.gpsimd.dma_start(t[:], wo_d[h * P:(h + 1) * P, :])
                wo_sb.append(t)

            kT_sb = kvp.tile([P, S], bf16, tag="kT")   # one kv head
            v_sb = [vp.tile([P, D], bf16, tag="v", name=f"v{i}")
                    for i in range(NSC)]

            def rope_evac(dst, ps, j, tag):
                """dst (bf16) = rope(ps) at abs position j*BS.

                ps: [d, BS] f32 PSUM projection. Uses one PE perm-matmul for
                rotate-half, then DVE combines with cos/sin."""
                raw = rawp.tile([P, BS], bf16, tag="raw")
                nc.scalar.copy(raw[:], ps[:])
                rot = psP.tile([P, BS], f32, tag="p")
                nc.tensor.matmul(rot[:], perm_sb[:], raw[:],
                                 start=True, stop=True)
                cs = cos_sb[:, j * BS:(j + 1) * BS]
                sn = sin_sb[:, j * BS:(j + 1) * BS]
                tm = rtp.tile([P, BS], bf16, tag="rt")
                nc.vector.tensor_mul(tm[:], raw[:], cs)
                t2 = rtp.tile([P, BS], bf16, tag="rt")
                nc.vector.tensor_mul(t2[:], rot[:], sn)
                nc.vector.tensor_add(dst, tm[:], t2[:])

            for j in range(NB):
                js = slice(j * BS, (j + 1) * BS)
                # ---- xT panel (bf16, pure HW DMA) ----
                xt = []
                for e in range(NE):
                    t = xtp.tile([P, BS], bf16, tag="xt")
                    nc.sync.dma_start(t[:], xT_d[e * P:(e + 1) * P, js])
                    xt.append(t)

                # ---- projections ----
                ps = psP.tile([P, BS], f32, tag="p")
                for e in range(NE):
                    nc.tensor.matmul(ps[:], wk_sb[e][:], xt[e][:],
                                     start=(e == 0), stop=(e == NE - 1))
                rope_evac(kT_sb[:, js], ps, j, "k")

                qT = []
                for h in range(G):
                    ps = psP.tile([P, BS], f32, tag="p")
                    for e in range(NE):
                        nc.tensor.matmul(ps[:], wq_sb[e][:, h * D:(h + 1) * D],
                                         xt[e][:],
                                         start=(e == 0), stop=(e == NE - 1))
                    qh = qtp.tile([P, BS], bf16, tag="qT")
                    rope_evac(qh[:], ps, j, f"q{h}")
                    qT.append(qh)

                for sc in range(4):
                    scg = 4 * j + sc          # global s-chunk
                    ps = psP.tile([P, D], f32, tag="p")
                    for e in range(NE):
                        nc.tensor.matmul(
                            ps[:], xt[e][:, sc * P:(sc + 1) * P], wv_sb[e][:],
                            start=(e == 0), stop=(e == NE - 1))
                    nc.vector.tensor_copy(v_sb[scg][:], ps[:])

                # ---- attention ----
                nt = 4 * j + 4
                oraw, rsv = [], []
                for h in range(G):
                    outp = psO.tile([P, BS], f32, tag="o")
                    exs = esp.tile([P, BS], bf16, tag="es")
                    for t in range(nt):
                        off = (t - 4 * j) * P if t >= 4 * j else 0
                        sp = psS.tile([P, BS], f32, tag="s")
                        nc.tensor.matmul(sp[:, off:], kT_sb[:, t * P:(t + 1) * P],
                                         qT[h][:, off:], start=True, stop=True)
                        ex = exp_p.tile([P, BS], bf16, tag="ex")
                        nc.scalar.activation(ex[:, off:], sp[:, off:], EXP,
                                             scale=SCALE)
                        if t >= 4 * j:
                            nc.vector.tensor_mul(ex[:, off:off + P],
                                                 ex[:, off:off + P], tri_sb[:])
                        if t == 0:
                            nc.vector.tensor_copy(exs[:], ex[:])
                        else:
                            nc.vector.tensor_add(exs[:, off:], exs[:, off:],
                                                 ex[:, off:])
                        nc.tensor.matmul(outp[:, off:], v_sb[t][:], ex[:, off:],
                                         start=(t == 0), stop=(t == nt - 1),
                                         skip_group_check=(off > 0))
                    rs = psO.tile([1, BS], f32, tag="o")
                    nc.tensor.matmul(rs[:], ones_col[:], exs[:],
                                     start=True, stop=True)
                    rv = recp.tile([1, BS], f32, tag="rsv")
                    nc.scalar.copy(rv[:], rs[:])   # Copy is in every ACT set
                    rsv.append(rv)
                    orw = otp.tile([P, BS], bf16, tag="orw")
                    nc.scalar.copy(orw[:], outp[:])
                    oraw.append(orw)
                # batched 1/x = exp(-ln(x)): 4 Ln then 4 Exp keeps ACT
                # table swaps to 2 per block instead of 2 per head
                lgs = []
                for h in range(G):
                    lg = recp.tile([1, BS], f32, tag="lg")
                    nc.scalar.activation(lg[:], rsv[h][:], LN)
                    lgs.append(lg)
                recs = []
                for h in range(G):
                    rec = recp.tile([1, BS], bf16, tag="rec")
                    nc.scalar.activation(rec[:], lgs[h][:], EXP, scale=-1.0)
                    recs.append(rec)
                outT = []
                for h in range(G):
                    rb = psO.tile([P, BS], f32, tag="o")
                    nc.tensor.matmul(rb[:], ones_row[:], recs[h][:],
                                     start=True, stop=True)
                    ot = otp.tile([P, BS], bf16, tag="oT")
                    nc.vector.tensor_mul(ot[:], oraw[h][:], rb[:])
                    outT.append(ot)

                # ---- output projection ----
                for sc in range(4):
                    yb = ybp.tile([P, E], bf16, tag="y")
                    for eb in range(4):
                        ypn = psY.tile([P, BS], f32, tag="y")
                        for h in range(G):
                            nc.tensor.matmul(
                                ypn[:],
                                outT[h][:, sc * P:(sc + 1) * P],
                                wo_sb[h][:, eb * BS:(eb + 1) * BS],
                                start=(h == 0), stop=(h == G - 1))
                        if eb % 2 == 0:
                            nc.scalar.copy(yb[:, eb * BS:(eb + 1) * BS], ypn[:])
                        else:
                            nc.vector.tensor_copy(yb[:, eb * BS:(eb + 1) * BS],
                                                  ypn[:])
                    r0 = j * BS + sc * P
                    nc.gpsimd.dma_start(y_d[r0:r0 + P, :], yb[:])

    nc.compile()
    _CACHE["nc"] = nc
    return nc


def _tables():
    inv = 1.0 / THETA ** (np.arange(0, D, 2, dtype=np.float64) / D)   # [64]
    t = np.arange(S, dtype=np.float64)
    fr = np.outer(inv, t)                    # [64, S]
    cosT = np.empty((P, S), dtype=np.float32)
    cosT[0:64] = np.cos(fr)
    cosT[64:128] = np.cos(fr)
    sinT = np.empty((P, S), dtype=np.float32)
    sinT[0:64] = np.sin(fr)
    sinT[64:128] = np.sin(fr)
    # tri[p, c] = 1 if p <= c (valid) else 0 — the causal boundary block
    tri = (np.arange(P)[:, None] <= np.arange(P)[None, :]).astype(np.float32)
    # perm as lhsT: rot = perm.T @ q -> rot[i] = -q[i+64] (i<64), q[i-64] (i>=64)
    perm = np.zeros((P, P), dtype=np.float32)
    perm[np.arange(64) + 64, np.arange(64)] = -1.0
    perm[np.arange(64), np.arange(64) + 64] = 1.0
    return cosT.astype(BF), sinT, tri.astype(BF), perm.astype(BF)


def _in_maps(x, Wq, Wk, Wv, Wo):
    cosT, sinT, tri, perm = _tables()
    xT = [np.ascontiguousarray(x[b].T.astype(BF)) for b in range(B)]
    wq = [np.ascontiguousarray(Wq[:, kv * G * D:(kv + 1) * G * D].astype(BF))
          for kv in range(KV)]
    wk = [np.ascontiguousarray(Wk[:, kv * D:(kv + 1) * D].astype(BF))
          for kv in range(KV)]
    wv = [np.ascontiguousarray(Wv[:, kv * D:(kv + 1) * D].astype(BF))
          for kv in range(KV)]
    wo = [np.ascontiguousarray(Wo[kv * G * D:(kv + 1) * G * D, :].astype(BF))
          for kv in range(KV)]
    maps = []
    for c in range(8):
        b, kv = c // 4, c % 4
        maps.append({
            "xT": xT[b], "wq": wq[kv], "wk": wk[kv], "wv": wv[kv],
            "wo": wo[kv], "cosT": cosT, "sinT": sinT, "tri": tri,
            "perm": perm,
        })
    return maps


def _gather(results):
    out = np.empty((B, S, E), dtype=np.float32)
    for b in range(B):
        acc = results[4 * b]["y"].astype(np.float32)
        for kv in range(1, 4):
            acc += results[4 * b + kv]["y"].astype(np.float32)
        out[b] = acc
    return out


def run(x, Wq, Wk, Wv, Wo, trace=False, **trace_kwargs):
    from concourse.bass_utils import run_bass_kernel_spmd
    nc = _build()
    res = run_bass_kernel_spmd(nc, _in_maps(x, Wq, Wk, Wv, Wo),
                               list(range(8)), trace=trace, **trace_kwargs)
    return _gather(res.results), res


def kernel(x, Wq, Wk, Wv, Wo):
    out, _ = run(np.asarray(x), np.asarray(Wq), np.asarray(Wk),
                 np.asarray(Wv), np.asarray(Wo))
    return out


# revision 11
# speedup vs baseline: 1.2374x; 1.2374x over previous
"""GQA attention (B=2,S=2048,E=2048,H=16,KV=4,D=128, RoPE, causal) on 8 trn2 cores.

Sharding: core c = (b = c//4, kv = c%4). Tensor-parallel over kv-head groups
(Wq cols / Wk,Wv cols / Wo rows) x data-parallel over batch. Each core computes
a full [S, E] partial output (its head group's contribution) in bf16; host sums
the 4 partials per batch element in f32.

All inputs are cast to bf16 on the HOST (no cast DMAs on device). Layout:
  qT/kT [d, s] = Wq_chunk.T @ xT   (PSUM accum over e-chunks)
  rot(q) via a single 128x128 permutation matmul (PermT.T @ qraw), then
  rope = qraw*cos + rot*sin on DVE (one extra matmul instead of a second
  full projection).
  v     [s, d] = xT_chunk.T @ Wv
  scoresT [sk, sq] = kT_chunk.T @ qT_block -> exp (no max-subtraction; scores
     are ~N(0,0.8)). Causal handling: strips above the block-diagonal are
     skipped; diagonal strips compute only the valid column suffix, and the
     single 128x128 boundary block is masked with a triangular 0/1 multiply.
  rowsum: DVE accumulates exp tiles (bf16) -> one ones-matmul per (h, block)
     -> 1/x via ACT exp(-log(x)) -> broadcast with a K=1 matmul.
  outT [d, sq] += v_chunk.T @ expT  (PSUM accum over sk-chunks)
  y [sq, e] += outT_norm_chunk.T @ Wo_head  (accum over 4 heads), bf16 out.
"""
import sys
sys.path.insert(0, "/opt/trn_rl_repo")
import numpy as np
import ml_dtypes

BF = ml_dtypes.bfloat16

B, S, E = 2, 2048, 2048
H, KV, D = 16, 4, 128
G = H // KV          # 4 q heads per kv head / core
THETA = 10000.0
P = 128
NE = E // P          # 16 e-chunks
NB = 4               # s-blocks per core loop
BS = S // NB         # 512
NSC = S // P         # 16 s-chunks

_CACHE = {}


def _build():
    if "nc" in _CACHE:
        return _CACHE["nc"]
    import concourse.bass as bass
    import concourse.tile as tile
    from concourse import mybir, bacc, bass_isa

    f32 = mybir.dt.float32
    bf16 = mybir.dt.bfloat16
    EXP = mybir.ActivationFunctionType.Exp
    LN = mybir.ActivationFunctionType.Ln
    SCALE = 1.0 / np.sqrt(D)

    nc = bacc.Bacc("TRN2", target_bir_lowering=False, debug=False)
    xT_d = nc.declare_dram_parameter("xT", [E, S], bf16, isOutput=False)
    wq_d = nc.declare_dram_parameter("wq", [E, G * D], bf16, isOutput=False)
    wk_d = nc.declare_dram_parameter("wk", [E, D], bf16, isOutput=False)
    wv_d = nc.declare_dram_parameter("wv", [E, D], bf16, isOutput=False)
    wo_d = nc.declare_dram_parameter("wo", [G * D, E], bf16, isOutput=False)
    cos_d = nc.declare_dram_parameter("cosT", [P, S], bf16, isOutput=False)
    sin_d = nc.declare_dram_parameter("sinT", [P, S], f32, isOutput=False)
    tri_d = nc.declare_dram_parameter("tri", [P, P], bf16, isOutput=False)
    perm_d = nc.declare_dram_parameter("perm", [P, P], bf16, isOutput=False)
    y_d = nc.declare_dram_parameter("y", [S, E], bf16, isOutput=True)

    with tile.TileContext(nc) as tc, \
         nc.allow_low_precision(reason="bf16 matmul pipeline"):
        import contextlib
        with contextlib.ExitStack() as ctx:
            cst = ctx.enter_context(tc.tile_pool(name="cst", bufs=1))
            wqp = ctx.enter_context(tc.tile_pool(name="wqp", bufs=16))
            wkvp = ctx.enter_context(tc.tile_pool(name="wkvp", bufs=32))
            wop = ctx.enter_context(tc.tile_pool(name="wop", bufs=4))
            xtp = ctx.enter_context(tc.tile_pool(name="xtp", bufs=48))
            kvp = ctx.enter_context(tc.tile_pool(name="kvp", bufs=1))
            vp = ctx.enter_context(tc.tile_pool(name="vp", bufs=16))
            qtp = ctx.enter_context(tc.tile_pool(name="qtp", bufs=8))
            rawp = ctx.enter_context(tc.tile_pool(name="rawp", bufs=4))
            rtp = ctx.enter_context(tc.tile_pool(name="rtp", bufs=6))
            exp_p = ctx.enter_context(tc.tile_pool(name="exp", bufs=6))
            esp = ctx.enter_context(tc.tile_pool(name="esp", bufs=2))
            recp = ctx.enter_context(tc.tile_pool(name="recp", bufs=4))
            otp = ctx.enter_context(tc.tile_pool(name="otp", bufs=8))
            ybp = ctx.enter_context(tc.tile_pool(name="ybp", bufs=3))
            psA = ctx.enter_context(tc.tile_pool(name="psA", bufs=3, space="PSUM"))
            psO = ctx.enter_context(tc.tile_pool(name="psO", bufs=2, space="PSUM"))
            psY = ctx.enter_context(tc.tile_pool(name="psY", bufs=2, space="PSUM"))
            psRB = ctx.enter_context(tc.tile_pool(name="psRB", bufs=1, space="PSUM"))

            # ---- constants / weights (resident) ----
            cos_sb = cst.tile([P, S], bf16, tag="cos")
            sin_sb = cst.tile([P, S], f32, tag="sin")
            tri_sb = cst.tile([P, P], bf16, tag="tri")
            perm_sb = cst.tile([P, P], bf16, tag="perm")
            nc.sync.dma_start(cos_sb[:], cos_d[:])
            nc.sync.dma_start(sin_sb[:], sin_d[:])
            nc.gpsimd.dma_start(tri_sb[:], tri_d[:])
            nc.gpsimd.dma_start(perm_sb[:], perm_d[:])
            ones_col = cst.tile([P, 1], bf16, tag="onc")
            nc.vector.memset(ones_col[:], 1.0)
            ones_row = cst.tile([1, P], bf16, tag="onr")
            nc.vector.memset(ones_row[:], 1.0)

            wk_sb, wv_sb = [], []
            for e in range(NE):
                t = wkvp.tile([P, D], bf16, tag="wk")
                nc.gpsimd.dma_start(t[:], wk_d[e * P:(e + 1) * P, :])
                wk_sb.append(t)
                t = wkvp.tile([P, D], bf16, tag="wv")
                nc.gpsimd.dma_start(t[:], wv_d[e * P:(e + 1) * P, :])
                wv_sb.append(t)
            wq_sb = []
            for e in range(NE):
                t = wqp.tile([P, G * D], bf16, tag="wq")
                nc.gpsimd.dma_start(t[:], wq_d[e * P:(e + 1) * P, :])
                wq_sb.append(t)
            wo_sb = []
            for h in range(G):
                t = wop.tile([P, E], bf16, tag="wo")
                nc.gpsimd.dma_start(t[:], wo_d[h * P:(h + 1) * P, :])
                wo_sb.append(t)

            kT_sb = kvp.tile([P, S], bf16, tag="kT")   # one kv head
            v_sb = [vp.tile([P, D], bf16, tag="v", name=f"v{i}")
                    for i in range(NSC)]

            def rope_evac(dst, ps, j, tag):
                """dst (bf16) = rope(ps) at abs position j*BS.

                ps: [d, BS] f32 PSUM projection. Uses one PE perm-matmul for
                rotate-half, then DVE combines with cos/sin."""
                raw = rawp.tile([P, BS], bf16, tag="raw")
                nc.scalar.copy(raw[:], ps[:])
                rot = psA.tile([P, BS], f32, tag="a")
                nc.tensor.matmul(rot[:], perm_sb[:], raw[:],
                                 start=True, stop=True)
                cs = cos_sb[:, j * BS:(j + 1) * BS]
                sn = sin_sb[:, j * BS:(j + 1) * BS]
                tm = rtp.tile([P, BS], bf16, tag="rt")
                nc.vector.tensor_mul(tm[:], raw[:], cs)
                t2 = rtp.tile([P, BS], bf16, tag="rt")
                nc.vector.tensor_mul(t2[:], rot[:], sn)
                nc.vector.tensor_add(dst, tm[:], t2[:])

            for j in range(NB):
                js = slice(j * BS, (j + 1) * BS)
                # ---- xT panel (bf16, pure HW DMA) ----
                xt = []
                for e in range(NE):
                    t = xtp.tile([P, BS], bf16, tag="xt")
                    nc.sync.dma_start(t[:], xT_d[e * P:(e + 1) * P, js])
                    xt.append(t)

                # ---- projections ----
                ps = psA.tile([P, BS], f32, tag="a")
                for e in range(NE):
                    nc.tensor.matmul(ps[:], wk_sb[e][:], xt[e][:],
                                     start=(e == 0), stop=(e == NE - 1))
                rope_evac(kT_sb[:, js], ps, j, "k")

                qT = []
                for h in range(G):
                    ps = psA.tile([P, BS], f32, tag="a")
                    for e in range(NE):
                        nc.tensor.matmul(ps[:], wq_sb[e][:, h * D:(h + 1) * D],
                                         xt[e][:],
                                         start=(e == 0), stop=(e == NE - 1))
                    qh = qtp.tile([P, BS], bf16, tag="qT")
                    rope_evac(qh[:], ps, j, f"q{h}")
                    qT.append(qh)

                for sc in range(4):
                    scg = 4 * j + sc          # global s-chunk
                    ps = psA.tile([P, D], f32, tag="a")
                    for e in range(NE):
                        nc.tensor.matmul(
                            ps[:], xt[e][:, sc * P:(sc + 1) * P], wv_sb[e][:],
                            start=(e == 0), stop=(e == NE - 1))
                    nc.scalar.copy(v_sb[scg][:], ps[:])

                # ---- attention ----
                nt = 4 * j + 4
                oraw, rsv = [], []
                for h in range(G):
                    outp = psO.tile([P, BS], f32, tag="o")
                    exs = esp.tile([P, BS], bf16, tag="es")
                    for t in range(nt):
                        off = (t - 4 * j) * P if t >= 4 * j else 0
                        sp = psA.tile([P, BS], f32, tag="a")
                        nc.tensor.matmul(sp[:, off:], kT_sb[:, t * P:(t + 1) * P],
                                         qT[h][:, off:], start=True, stop=True)
                        ex = exs if t == 0 else exp_p.tile([P, BS], bf16,
                                                            tag="ex")
                        nc.scalar.activation(ex[:, off:], sp[:, off:], EXP,
                                             scale=SCALE)
                        if t >= 4 * j:
                            nc.vector.tensor_mul(ex[:, off:off + P],
                                                 ex[:, off:off + P], tri_sb[:])
                        if t > 0:
                            nc.vector.tensor_add(exs[:, off:], exs[:, off:],
                                                 ex[:, off:])
                        nc.tensor.matmul(outp[:, off:], v_sb[t][:], ex[:, off:],
                                         start=(t == 0), stop=(t == nt - 1),
                                         skip_group_check=(off > 0))
                    rs = psRB.tile([1, BS], f32, tag="r")
                    nc.tensor.matmul(rs[:], ones_col[:], exs[:],
                                     start=True, stop=True)
                    rv = recp.tile([1, BS], f32, tag="rsv")
                    nc.scalar.copy(rv[:], rs[:])   # Copy is in every ACT set
                    rsv.append(rv)
                    orw = otp.tile([P, BS], bf16, tag="orw")
                    nc.scalar.copy(orw[:], outp[:])
                    oraw.append(orw)
                # batched 1/x = exp(-ln(x)): 4 Ln then 4 Exp keeps ACT
                # table swaps to 2 per block instead of 2 per head
                lgs = []
                for h in range(G):
                    lg = recp.tile([1, BS], f32, tag="lg")
                    nc.scalar.activation(lg[:], rsv[h][:], LN)
                    lgs.append(lg)
                recs = []
                for h in range(G):
                    rec = recp.tile([1, BS], bf16, tag="rec")
                    nc.scalar.activation(rec[:], lgs[h][:], EXP, scale=-1.0)
                    recs.append(rec)
                outT = []
                for h in range(G):
                    rb = psO.tile([P, BS], f32, tag="o")
                    nc.tensor.matmul(rb[:], ones_row[:], recs[h][:],
                                     start=True, stop=True)
                    ot = otp.tile([P, BS], bf16, tag="oT")
                    nc.vector.tensor_mul(ot[:], oraw[h][:], rb[:])
                    outT.append(ot)

                # ---- output projection ----
                for sc in range(4):
                    yb = ybp.tile([P, E], bf16, tag="y")
                    for eb in range(4):
                        ypn = psY.tile([P, BS], f32, tag="y")
                        for h in range(G):
                            nc.tensor.matmul(
                                ypn[:],
                                outT[h][:, sc * P:(sc + 1) * P],
                                wo_sb[h][:, eb * BS:(eb + 1) * BS],
                                start=(h == 0), stop=(h == G - 1))
                        if eb % 2 == 0:
                            nc.scalar.copy(yb[:, eb * BS:(eb + 1) * BS], ypn[:])
                        else:
                            nc.vector.tensor_copy(yb[:, eb * BS:(eb + 1) * BS],
                                                  ypn[:])
                    r0 = j * BS + sc * P
                    nc.gpsimd.dma_start(y_d[r0:r0 + P, :], yb[:])

    nc.compile()
    _CACHE["nc"] = nc
    return nc


def _tables():
    inv = 1.0 / THETA ** (np.arange(0, D, 2, dtype=np.float64) / D)   # [64]
    t = np.arange(S, dtype=np.float64)
    fr = np.outer(inv, t)                    # [64, S]
    cosT = np.empty((P, S), dtype=np.float32)
    cosT[0:64] = np.cos(fr)
    cosT[64:128] = np.cos(fr)
    sinT = np.empty((P, S), dtype=np.float32)
    sinT[0:64] = np.sin(fr)
    sinT[64:128] = np.sin(fr)
    # tri[p, c] = 1 if p <= c (valid) else 0 — the causal boundary block
    tri = (np.arange(P)[:, None] <= np.arange(P)[None, :]).astype(np.float32)
    # perm as lhsT: rot = perm.T @ q -> rot[i] = -q[i+64] (i<64), q[i-64] (i>=64)
    perm = np.zeros((P, P), dtype=np.float32)
    perm[np.arange(64) + 64, np.arange(64)] = -1.0
    perm[np.arange(64), np.arange(64) + 64] = 1.0
    return cosT.astype(BF), sinT, tri.astype(BF), perm.astype(BF)


def _in_maps(x, Wq, Wk, Wv, Wo):
    cosT, sinT, tri, perm = _tables()
    xT = [np.ascontiguousarray(x[b].T.astype(BF)) for b in range(B)]
    wq = [np.ascontiguousarray(Wq[:, kv * G * D:(kv + 1) * G * D].astype(BF))
          for kv in range(KV)]
    wk = [np.ascontiguousarray(Wk[:, kv * D:(kv + 1) * D].astype(BF))
          for kv in range(KV)]
    wv = [np.ascontiguousarray(Wv[:, kv * D:(kv + 1) * D].astype(BF))
          for kv in range(KV)]
    wo = [np.ascontiguousarray(Wo[kv * G * D:(kv + 1) * G * D, :].astype(BF))
          for kv in range(KV)]
    maps = []
    for c in range(8):
        b, kv = c // 4, c % 4
        maps.append({
            "xT": xT[b], "wq": wq[kv], "wk": wk[kv], "wv": wv[kv],
            "wo": wo[kv], "cosT": cosT, "sinT": sinT, "tri": tri,
            "perm": perm,
        })
    return maps


def _gather(results):
    out = np.empty((B, S, E), dtype=np.float32)
    for b in range(B):
        acc = results[4 * b]["y"].astype(np.float32)
        for kv in range(1, 4):
            acc += results[4 * b + kv]["y"].astype(np.float32)
        out[b] = acc
    return out


def run(x, Wq, Wk, Wv, Wo, trace=False, **trace_kwargs):
    from concourse.bass_utils import run_bass_kernel_spmd
    nc = _build()
    res = run_bass_kernel_spmd(nc, _in_maps(x, Wq, Wk, Wv, Wo),
                               list(range(8)), trace=trace, **trace_kwargs)
    return _gather(res.results), res


def kernel(x, Wq, Wk, Wv, Wo):
    out, _ = run(np.asarray(x), np.asarray(Wq), np.asarray(Wk),
                 np.asarray(Wv), np.asarray(Wo))
    return out


# revision 12
# speedup vs baseline: 1.3807x; 1.1158x over previous
"""GQA attention (B=2,S=2048,E=2048,H=16,KV=4,D=128, RoPE, causal) on 8 trn2 cores.

Sharding: core c = (b = c//4, kv = c%4). Tensor-parallel over kv-head groups
(Wq cols / Wk,Wv cols / Wo rows) x data-parallel over batch. Each core computes
a full [S, E] partial output (its head group's contribution) in bf16; host sums
the 4 partials per batch element in f32.

All inputs are cast to bf16 on the HOST (no cast DMAs on device). Layout:
  qT/kT [d, s] = Wq_chunk.T @ xT   (PSUM accum over e-chunks)
  rot(q) via a single 128x128 permutation matmul, then rope = raw*cos +
  rot*sin on DVE (one extra matmul instead of a second full projection).
  v     [s, d] = xT_chunk.T @ Wv
  scoresT [sk, sq] = kT_chunk.T @ qT_block -> exp (no max-subtraction; scores
     are ~N(0,0.8)). Causal handling: strips above the block-diagonal are
     skipped; diagonal strips compute only the valid column suffix, and the
     single 128x128 boundary block is masked with a triangular 0/1 multiply.
  rowsum: DVE accumulates exp tiles (bf16, t=0 exp writes the accumulator
     directly) -> one ones-matmul per (h, block) -> 1/x = exp(-ln(x)) on ACT
     (Ln/Exp batched per block: 2 table swaps per block, copies are free)
     -> broadcast with a K=1 matmul.
  outT [d, sq] += v_chunk.T @ expT  (PSUM accum over sk-chunks)
  y [sq, e] += outT_norm_chunk.T @ Wo_head  (accum over 4 heads), bf16 out.

Schedule: ~6us of zero-matmuls at kernel start warm the PE HAM clock while
weight DMAs land; projections for block j+1 are emitted between attention(j)
and the normalize/yproj tail so the PE has fill work during the softmax
normalization latency at block boundaries.
"""
import sys
sys.path.insert(0, "/opt/trn_rl_repo")
import numpy as np
import ml_dtypes

BF = ml_dtypes.bfloat16

B, S, E = 2, 2048, 2048
H, KV, D = 16, 4, 128
G = H // KV          # 4 q heads per kv head / core
THETA = 10000.0
P = 128
NE = E // P          # 16 e-chunks
NB = 4               # s-blocks per core loop
BS = S // NB         # 512
NSC = S // P         # 16 s-chunks

_CACHE = {}


def _build():
    if "nc" in _CACHE:
        return _CACHE["nc"]
    import concourse.bass as bass
    import concourse.tile as tile
    from concourse import mybir, bacc

    f32 = mybir.dt.float32
    bf16 = mybir.dt.bfloat16
    EXP = mybir.ActivationFunctionType.Exp
    LN = mybir.ActivationFunctionType.Ln
    SCALE = 1.0 / np.sqrt(D)

    nc = bacc.Bacc("TRN2", target_bir_lowering=False, debug=False)
    xT_d = nc.declare_dram_parameter("xT", [E, S], bf16, isOutput=False)
    wq_d = nc.declare_dram_parameter("wq", [E, G * D], bf16, isOutput=False)
    wk_d = nc.declare_dram_parameter("wk", [E, D], bf16, isOutput=False)
    wv_d = nc.declare_dram_parameter("wv", [E, D], bf16, isOutput=False)
    wo_d = nc.declare_dram_parameter("wo", [G * D, E], bf16, isOutput=False)
    cos_d = nc.declare_dram_parameter("cosT", [P, S], bf16, isOutput=False)
    sin_d = nc.declare_dram_parameter("sinT", [P, S], f32, isOutput=False)
    tri_d = nc.declare_dram_parameter("tri", [P, P], bf16, isOutput=False)
    perm_d = nc.declare_dram_parameter("perm", [P, P], bf16, isOutput=False)
    y_d = nc.declare_dram_parameter("y", [S, E], bf16, isOutput=True)

    with tile.TileContext(nc) as tc, \
         nc.allow_low_precision(reason="bf16 matmul pipeline"):
        import contextlib
        with contextlib.ExitStack() as ctx:
            cst = ctx.enter_context(tc.tile_pool(name="cst", bufs=1))
            wqp = ctx.enter_context(tc.tile_pool(name="wqp", bufs=16))
            wkvp = ctx.enter_context(tc.tile_pool(name="wkvp", bufs=32))
            wop = ctx.enter_context(tc.tile_pool(name="wop", bufs=4))
            xtp = ctx.enter_context(tc.tile_pool(name="xtp", bufs=48))
            kvp = ctx.enter_context(tc.tile_pool(name="kvp", bufs=1))
            vp = ctx.enter_context(tc.tile_pool(name="vp", bufs=16))
            qtp = ctx.enter_context(tc.tile_pool(name="qtp", bufs=8))
            rawp = ctx.enter_context(tc.tile_pool(name="rawp", bufs=4))
            rtp = ctx.enter_context(tc.tile_pool(name="rtp", bufs=6))
            exp_p = ctx.enter_context(tc.tile_pool(name="exp", bufs=6))
            esp = ctx.enter_context(tc.tile_pool(name="esp", bufs=2))
            recp = ctx.enter_context(tc.tile_pool(name="recp", bufs=4))
            otp = ctx.enter_context(tc.tile_pool(name="otp", bufs=8))
            ybp = ctx.enter_context(tc.tile_pool(name="ybp", bufs=3))
            psA = ctx.enter_context(tc.tile_pool(name="psA", bufs=3, space="PSUM"))
            psO = ctx.enter_context(tc.tile_pool(name="psO", bufs=2, space="PSUM"))
            psY = ctx.enter_context(tc.tile_pool(name="psY", bufs=2, space="PSUM"))
            psRB = ctx.enter_context(tc.tile_pool(name="psRB", bufs=1, space="PSUM"))

            # ---- HAM warmup: ~6us of dependency-free matmuls keep the PE
            # clock-gate busy while the weight DMAs land ----
            wz = cst.tile([P, P], bf16, tag="wz")
            nc.vector.memset(wz[:], 0.0)
            wps = psRB.tile([P, BS], f32, tag="r")
            for _ in range(56):
                nc.tensor.matmul(wps[:, :P], wz[:], wz[:], start=True,
                                 stop=True)

            # ---- constants / weights (resident) ----
            cos_sb = cst.tile([P, S], bf16, tag="cos")
            sin_sb = cst.tile([P, S], f32, tag="sin")
            tri_sb = cst.tile([P, P], bf16, tag="tri")
            perm_sb = cst.tile([P, P], bf16, tag="perm")
            ones_col = cst.tile([P, 1], bf16, tag="onc")
            nc.vector.memset(ones_col[:], 1.0)
            ones_row = cst.tile([1, P], bf16, tag="onr")
            nc.vector.memset(ones_row[:], 1.0)

            # first-needed first: wk/wq on gpsimd, xt(0) on sync (emitted in
            # emit_proj(0) below), consts on the scalar queue, then wv/wo
            wk_sb = []
            for e in range(NE):
                t = wkvp.tile([P, D], bf16, tag="wk")
                nc.gpsimd.dma_start(t[:], wk_d[e * P:(e + 1) * P, :])
                wk_sb.append(t)
            wq_sb = []
            for e in range(NE):
                t = wqp.tile([P, G * D], bf16, tag="wq")
                nc.gpsimd.dma_start(t[:], wq_d[e * P:(e + 1) * P, :])
                wq_sb.append(t)
            nc.scalar.dma_start(cos_sb[:], cos_d[:])
            nc.scalar.dma_start(sin_sb[:], sin_d[:])
            nc.scalar.dma_start(tri_sb[:], tri_d[:])
            nc.scalar.dma_start(perm_sb[:], perm_d[:])
            wv_sb = []
            for e in range(NE):
                t = wkvp.tile([P, D], bf16, tag="wv")
                nc.gpsimd.dma_start(t[:], wv_d[e * P:(e + 1) * P, :])
                wv_sb.append(t)
            wo_sb = []
            for h in range(G):
                t = wop.tile([P, E], bf16, tag="wo")
                nc.gpsimd.dma_start(t[:], wo_d[h * P:(h + 1) * P, :])
                wo_sb.append(t)

            kT_sb = kvp.tile([P, S], bf16, tag="kT")   # one kv head
            v_sb = [vp.tile([P, D], bf16, tag="v", name=f"v{i}")
                    for i in range(NSC)]

            def rope_evac(dst, ps, j):
                """dst (bf16) = rope(ps) at abs position j*BS.

                ps: [d, BS] f32 PSUM projection. One PE perm-matmul for
                rotate-half, then DVE combines with cos/sin."""
                raw = rawp.tile([P, BS], bf16, tag="raw")
                nc.scalar.copy(raw[:], ps[:])
                rot = psA.tile([P, BS], f32, tag="a")
                nc.tensor.matmul(rot[:], perm_sb[:], raw[:],
                                 start=True, stop=True)
                cs = cos_sb[:, j * BS:(j + 1) * BS]
                sn = sin_sb[:, j * BS:(j + 1) * BS]
                tm = rtp.tile([P, BS], bf16, tag="rt")
                nc.vector.tensor_mul(tm[:], raw[:], cs)
                t2 = rtp.tile([P, BS], bf16, tag="rt")
                nc.vector.tensor_mul(t2[:], rot[:], sn)
                nc.vector.tensor_add(dst, tm[:], t2[:])

            def emit_proj(j):
                """xt loads + K/Q/V projections + rope for block j."""
                js = slice(j * BS, (j + 1) * BS)
                xt = []
                for e in range(NE):
                    t = xtp.tile([P, BS], bf16, tag="xt")
                    nc.sync.dma_start(t[:], xT_d[e * P:(e + 1) * P, js])
                    xt.append(t)

                ps = psA.tile([P, BS], f32, tag="a")
                for e in range(NE):
                    nc.tensor.matmul(ps[:], wk_sb[e][:], xt[e][:],
                                     start=(e == 0), stop=(e == NE - 1))
                rope_evac(kT_sb[:, js], ps, j)

                qT = []
                for h in range(G):
                    ps = psA.tile([P, BS], f32, tag="a")
                    for e in range(NE):
                        nc.tensor.matmul(ps[:], wq_sb[e][:, h * D:(h + 1) * D],
                                         xt[e][:],
                                         start=(e == 0), stop=(e == NE - 1))
                    qh = qtp.tile([P, BS], bf16, tag="qT")
                    rope_evac(qh[:], ps, j)
                    qT.append(qh)

                for sc in range(4):
                    scg = 4 * j + sc          # global s-chunk
                    ps = psA.tile([P, D], f32, tag="a")
                    for e in range(NE):
                        nc.tensor.matmul(
                            ps[:], xt[e][:, sc * P:(sc + 1) * P], wv_sb[e][:],
                            start=(e == 0), stop=(e == NE - 1))
                    nc.scalar.copy(v_sb[scg][:], ps[:])
                return qT

            qT = emit_proj(0)
            for j in range(NB):
                # ---- attention ----
                nt = 4 * j + 4
                oraw, rsv = [], []
                for h in range(G):
                    outp = psO.tile([P, BS], f32, tag="o")
                    exs = esp.tile([P, BS], bf16, tag="es")
                    for t in range(nt):
                        off = (t - 4 * j) * P if t >= 4 * j else 0
                        sp = psA.tile([P, BS], f32, tag="a")
                        nc.tensor.matmul(sp[:, off:], kT_sb[:, t * P:(t + 1) * P],
                                         qT[h][:, off:], start=True, stop=True)
                        ex = exs if t == 0 else exp_p.tile([P, BS], bf16,
                                                           tag="ex")
                        nc.scalar.activation(ex[:, off:], sp[:, off:], EXP,
                                             scale=SCALE)
                        if t >= 4 * j:
                            nc.vector.tensor_mul(ex[:, off:off + P],
                                                 ex[:, off:off + P], tri_sb[:])
                        if t > 0:
                            nc.vector.tensor_add(exs[:, off:], exs[:, off:],
                                                 ex[:, off:])
                        nc.tensor.matmul(outp[:, off:], v_sb[t][:], ex[:, off:],
                                         start=(t == 0), stop=(t == nt - 1),
                                         skip_group_check=(off > 0))
                    rs = psRB.tile([1, BS], f32, tag="r")
                    nc.tensor.matmul(rs[:], ones_col[:], exs[:],
                                     start=True, stop=True)
                    rv = recp.tile([1, BS], f32, tag="rsv")
                    nc.scalar.copy(rv[:], rs[:])   # Copy is in every ACT set
                    rsv.append(rv)
                    orw = otp.tile([P, BS], bf16, tag="orw")
                    nc.scalar.copy(orw[:], outp[:])
                    oraw.append(orw)

                # projections for the next block fill the PE while the
                # normalize chain below resolves
                if j + 1 < NB:
                    qT_next = emit_proj(j + 1)

                # batched 1/x = exp(-ln(x)): 4 Ln then 4 Exp keeps ACT
                # table swaps to 2 per block instead of 2 per head
                lgs = []
                for h in range(G):
                    lg = recp.tile([1, BS], f32, tag="lg")
                    nc.scalar.activation(lg[:], rsv[h][:], LN)
                    lgs.append(lg)
                recs = []
                for h in range(G):
                    rec = recp.tile([1, BS], bf16, tag="rec")
                    nc.scalar.activation(rec[:], lgs[h][:], EXP, scale=-1.0)
                    recs.append(rec)
                outT = []
                for h in range(G):
                    rb = psO.tile([P, BS], f32, tag="o")
                    nc.tensor.matmul(rb[:], ones_row[:], recs[h][:],
                                     start=True, stop=True)
                    ot = otp.tile([P, BS], bf16, tag="oT")
                    nc.vector.tensor_mul(ot[:], oraw[h][:], rb[:])
                    outT.append(ot)

                # ---- output projection ----
                for sc in range(4):
                    yb = ybp.tile([P, E], bf16, tag="y")
                    for eb in range(4):
                        ypn = psY.tile([P, BS], f32, tag="y")
                        for h in range(G):
                            nc.tensor.matmul(
                                ypn[:],
                                outT[h][:, sc * P:(sc + 1) * P],
                                wo_sb[h][:, eb * BS:(eb + 1) * BS],
                                start=(h == 0), stop=(h == G - 1))
                        if eb % 2 == 0:
                            nc.scalar.copy(yb[:, eb * BS:(eb + 1) * BS], ypn[:])
                        else:
                            nc.vector.tensor_copy(yb[:, eb * BS:(eb + 1) * BS],
                                                  ypn[:])
                    r0 = j * BS + sc * P
                    nc.gpsimd.dma_start(y_d[r0:r0 + P, :], yb[:])

                if j + 1 < NB:
                    qT = qT_next

    nc.compile()
    _CACHE["nc"] = nc
    return nc


def _tables():
    inv = 1.0 / THETA ** (np.arange(0, D, 2, dtype=np.float64) / D)   # [64]
    t = np.arange(S, dtype=np.float64)
    fr = np.outer(inv, t)                    # [64, S]
    cosT = np.empty((P, S), dtype=np.float32)
    cosT[0:64] = np.cos(fr)
    cosT[64:128] = np.cos(fr)
    sinT = np.empty((P, S), dtype=np.float32)
    sinT[0:64] = np.sin(fr)
    sinT[64:128] = np.sin(fr)
    # tri[p, c] = 1 if p <= c (valid) else 0 — the causal boundary block
    tri = (np.arange(P)[:, None] <= np.arange(P)[None, :]).astype(np.float32)
    # perm as lhsT: rot = perm.T @ q -> rot[i] = -q[i+64] (i<64), q[i-64] (i>=64)
    perm = np.zeros((P, P), dtype=np.float32)
    perm[np.arange(64) + 64, np.arange(64)] = -1.0
    perm[np.arange(64), np.arange(64) + 64] = 1.0
    return cosT.astype(BF), sinT, tri.astype(BF), perm.astype(BF)


def _in_maps(x, Wq, Wk, Wv, Wo):
    cosT, sinT, tri, perm = _tables()
    xT = [np.ascontiguousarray(x[b].T.astype(BF)) for b in range(B)]
    wq = [np.ascontiguousarray(Wq[:, kv * G * D:(kv + 1) * G * D].astype(BF))
          for kv in range(KV)]
    wk = [np.ascontiguousarray(Wk[:, kv * D:(kv + 1) * D].astype(BF))
          for kv in range(KV)]
    wv = [np.ascontiguousarray(Wv[:, kv * D:(kv + 1) * D].astype(BF))
          for kv in range(KV)]
    wo = [np.ascontiguousarray(Wo[kv * G * D:(kv + 1) * G * D, :].astype(BF))
          for kv in range(KV)]
    maps = []
    for c in range(8):
        b, kv = c // 4, c % 4
        maps.append({
            "xT": xT[b], "wq": wq[kv], "wk": wk[kv], "wv": wv[kv],
            "wo": wo[kv], "cosT": cosT, "sinT": sinT, "tri": tri,
            "perm": perm,
        })
    return maps


def _gather(results):
    out = np.empty((B, S, E), dtype=np.float32)
    for b in range(B):
        acc = results[4 * b]["y"].astype(np.float32)
        for kv in range(1, 4):
            acc += results[4 * b + kv]["y"].astype(np.float32)
        out[b] = acc
    return out


def run(x, Wq, Wk, Wv, Wo, trace=False, **trace_kwargs):
    from concourse.bass_utils import run_bass_kernel_spmd
    nc = _build()
    res = run_bass_kernel_spmd(nc, _in_maps(x, Wq, Wk, Wv, Wo),
                               list(range(8)), trace=trace, **trace_kwargs)
    return _gather(res.results), res


def kernel(x, Wq, Wk, Wv, Wo):
    out, _ = run(np.asarray(x), np.asarray(Wq), np.asarray(Wk),
                 np.asarray(Wv), np.asarray(Wo))
    return out


# revision 14
# speedup vs baseline: 1.3870x; 1.0045x over previous
"""GQA attention (B=2,S=2048,E=2048,H=16,KV=4,D=128, RoPE, causal) on 8 trn2 cores.

Sharding: core c = (b = c//4, kv = c%4). Tensor-parallel over kv-head groups
(Wq cols / Wk,Wv cols / Wo rows) x data-parallel over batch. Each core computes
a full [S, E] partial output (its head group's contribution) in bf16; host sums
the 4 partials per batch element in f32.

All inputs are cast to bf16 on the HOST (no cast DMAs on device). Layout:
  qT/kT [d, s] = Wq_chunk.T @ xT   (PSUM accum over e-chunks)
  rot(q) via a single 128x128 permutation matmul, then rope = raw*cos +
  rot*sin on DVE (one extra matmul instead of a second full projection).
  v     [s, d] = xT_chunk.T @ Wv
  scoresT [sk, sq] = kT_chunk.T @ qT_block -> exp (no max-subtraction; scores
     are ~N(0,0.8)). Causal handling: strips above the block-diagonal are
     skipped; diagonal strips compute only the valid column suffix, and the
     single 128x128 boundary block is masked with a triangular 0/1 multiply.
  rowsum: DVE accumulates exp tiles (bf16, t=0 exp writes the accumulator
     directly) -> one ones-matmul per (h, block) -> 1/x = exp(-ln(x)) on ACT
     (Ln/Exp batched per block: 2 table swaps per block, copies are free)
     -> broadcast with a K=1 matmul.
  outT [d, sq] += v_chunk.T @ expT  (PSUM accum over sk-chunks)
  y [sq, e] += outT_norm_chunk.T @ Wo_head  (accum over 4 heads), bf16 out.

Schedule: ~6us of zero-matmuls at kernel start warm the PE HAM clock while
weight DMAs land; projections for block j+1 are emitted between attention(j)
and the normalize/yproj tail so the PE has fill work during the softmax
normalization latency at block boundaries.
"""
import sys
sys.path.insert(0, "/opt/trn_rl_repo")
import numpy as np
import ml_dtypes

BF = ml_dtypes.bfloat16

B, S, E = 2, 2048, 2048
H, KV, D = 16, 4, 128
G = H // KV          # 4 q heads per kv head / core
THETA = 10000.0
P = 128
NE = E // P          # 16 e-chunks
NB = 4               # s-blocks per core loop
BS = S // NB         # 512
NSC = S // P         # 16 s-chunks

_CACHE = {}


def _build():
    if "nc" in _CACHE:
        return _CACHE["nc"]
    import concourse.bass as bass
    import concourse.tile as tile
    from concourse import mybir, bacc

    f32 = mybir.dt.float32
    bf16 = mybir.dt.bfloat16
    EXP = mybir.ActivationFunctionType.Exp
    LN = mybir.ActivationFunctionType.Ln
    SCALE = 1.0 / np.sqrt(D)

    nc = bacc.Bacc("TRN2", target_bir_lowering=False, debug=False)
    xT_d = nc.declare_dram_parameter("xT", [E, S], bf16, isOutput=False)
    wq_d = nc.declare_dram_parameter("wq", [E, G * D], bf16, isOutput=False)
    wk_d = nc.declare_dram_parameter("wk", [E, D], bf16, isOutput=False)
    wv_d = nc.declare_dram_parameter("wv", [E, D], bf16, isOutput=False)
    wo_d = nc.declare_dram_parameter("wo", [G * D, E], bf16, isOutput=False)
    cos_d = nc.declare_dram_parameter("cosT", [P, S], bf16, isOutput=False)
    sin_d = nc.declare_dram_parameter("sinT", [P, S], f32, isOutput=False)
    tri_d = nc.declare_dram_parameter("tri", [P, P], bf16, isOutput=False)
    perm_d = nc.declare_dram_parameter("perm", [P, P], bf16, isOutput=False)
    y_d = nc.declare_dram_parameter("y", [S, E], bf16, isOutput=True)

    with tile.TileContext(nc) as tc, \
         nc.allow_low_precision(reason="bf16 matmul pipeline"):
        import contextlib
        with contextlib.ExitStack() as ctx:
            cst = ctx.enter_context(tc.tile_pool(name="cst", bufs=1))
            wqp = ctx.enter_context(tc.tile_pool(name="wqp", bufs=16))
            wkvp = ctx.enter_context(tc.tile_pool(name="wkvp", bufs=32))
            wop = ctx.enter_context(tc.tile_pool(name="wop", bufs=4))
            xtp = ctx.enter_context(tc.tile_pool(name="xtp", bufs=48))
            kvp = ctx.enter_context(tc.tile_pool(name="kvp", bufs=1))
            vp = ctx.enter_context(tc.tile_pool(name="vp", bufs=16))
            qtp = ctx.enter_context(tc.tile_pool(name="qtp", bufs=8))
            rawp = ctx.enter_context(tc.tile_pool(name="rawp", bufs=4))
            rtp = ctx.enter_context(tc.tile_pool(name="rtp", bufs=6))
            exp_p = ctx.enter_context(tc.tile_pool(name="exp", bufs=6))
            esp = ctx.enter_context(tc.tile_pool(name="esp", bufs=2))
            recp = ctx.enter_context(tc.tile_pool(name="recp", bufs=4))
            otp = ctx.enter_context(tc.tile_pool(name="otp", bufs=8))
            ybp = ctx.enter_context(tc.tile_pool(name="ybp", bufs=3))
            psA = ctx.enter_context(tc.tile_pool(name="psA", bufs=3, space="PSUM"))
            psO = ctx.enter_context(tc.tile_pool(name="psO", bufs=2, space="PSUM"))
            psY = ctx.enter_context(tc.tile_pool(name="psY", bufs=2, space="PSUM"))
            psRB = ctx.enter_context(tc.tile_pool(name="psRB", bufs=1, space="PSUM"))

            # ---- HAM warmup: ~6us of dependency-free matmuls keep the PE
            # clock-gate busy while the weight DMAs land ----
            wz = cst.tile([P, P], bf16, tag="wz")
            nc.vector.memset(wz[:], 0.0)
            wps = psRB.tile([P, BS], f32, tag="r")
            for _ in range(88):
                nc.tensor.matmul(wps[:, :P], wz[:], wz[:], start=True,
                                 stop=True)

            # ---- constants / weights (resident) ----
            cos_sb = cst.tile([P, S], bf16, tag="cos")
            sin_sb = cst.tile([P, S], f32, tag="sin")
            tri_sb = cst.tile([P, P], bf16, tag="tri")
            perm_sb = cst.tile([P, P], bf16, tag="perm")
            ones_col = cst.tile([P, 1], bf16, tag="onc")
            nc.vector.memset(ones_col[:], 1.0)
            ones_row = cst.tile([1, P], bf16, tag="onr")
            nc.vector.memset(ones_row[:], 1.0)

            # first-needed first: wk/wq on gpsimd, xt(0) on sync (emitted in
            # emit_proj(0) below), consts on the scalar queue, then wv/wo
            wk_sb = []
            for e in range(NE):
                t = wkvp.tile([P, D], bf16, tag="wk")
                nc.gpsimd.dma_start(t[:], wk_d[e * P:(e + 1) * P, :])
                wk_sb.append(t)
            wq_sb = []
            for e in range(NE):
                t = wqp.tile([P, G * D], bf16, tag="wq")
                nc.gpsimd.dma_start(t[:], wq_d[e * P:(e + 1) * P, :])
                wq_sb.append(t)
            nc.scalar.dma_start(cos_sb[:], cos_d[:])
            nc.scalar.dma_start(sin_sb[:], sin_d[:])
            nc.scalar.dma_start(tri_sb[:], tri_d[:])
            nc.scalar.dma_start(perm_sb[:], perm_d[:])
            wv_sb = []
            for e in range(NE):
                t = wkvp.tile([P, D], bf16, tag="wv")
                nc.gpsimd.dma_start(t[:], wv_d[e * P:(e + 1) * P, :])
                wv_sb.append(t)
            wo_sb = []
            for h in range(G):
                t = wop.tile([P, E], bf16, tag="wo")
                nc.gpsimd.dma_start(t[:], wo_d[h * P:(h + 1) * P, :])
                wo_sb.append(t)

            kT_sb = kvp.tile([P, S], bf16, tag="kT")   # one kv head
            v_sb = [vp.tile([P, D], bf16, tag="v", name=f"v{i}")
                    for i in range(NSC)]

            def rope_evac(dst, ps, j):
                """dst (bf16) = rope(ps) at abs position j*BS.

                ps: [d, BS] f32 PSUM projection. One PE perm-matmul for
                rotate-half, then DVE combines with cos/sin."""
                raw = rawp.tile([P, BS], bf16, tag="raw")
                nc.scalar.copy(raw[:], ps[:])
                rot = psA.tile([P, BS], f32, tag="a")
                nc.tensor.matmul(rot[:], perm_sb[:], raw[:],
                                 start=True, stop=True)
                cs = cos_sb[:, j * BS:(j + 1) * BS]
                sn = sin_sb[:, j * BS:(j + 1) * BS]
                tm = rtp.tile([P, BS], bf16, tag="rt")
                nc.vector.tensor_mul(tm[:], raw[:], cs)
                t2 = rtp.tile([P, BS], bf16, tag="rt")
                nc.vector.tensor_mul(t2[:], rot[:], sn)
                nc.vector.tensor_add(dst, tm[:], t2[:])

            def emit_proj(j):
                """xt loads + K/Q/V projections + rope for block j."""
                js = slice(j * BS, (j + 1) * BS)
                xt = []
                for e in range(NE):
                    t = xtp.tile([P, BS], bf16, tag="xt")
                    nc.sync.dma_start(t[:], xT_d[e * P:(e + 1) * P, js])
                    xt.append(t)

                ps = psA.tile([P, BS], f32, tag="a")
                for e in range(NE):
                    nc.tensor.matmul(ps[:], wk_sb[e][:], xt[e][:],
                                     start=(e == 0), stop=(e == NE - 1))
                rope_evac(kT_sb[:, js], ps, j)

                qT = []
                for h in range(G):
                    ps = psA.tile([P, BS], f32, tag="a")
                    for e in range(NE):
                        nc.tensor.matmul(ps[:], wq_sb[e][:, h * D:(h + 1) * D],
                                         xt[e][:],
                                         start=(e == 0), stop=(e == NE - 1))
                    qh = qtp.tile([P, BS], bf16, tag="qT")
                    rope_evac(qh[:], ps, j)
                    qT.append(qh)

                for sc in range(4):
                    scg = 4 * j + sc          # global s-chunk
                    ps = psA.tile([P, D], f32, tag="a")
                    for e in range(NE):
                        nc.tensor.matmul(
                            ps[:], xt[e][:, sc * P:(sc + 1) * P], wv_sb[e][:],
                            start=(e == 0), stop=(e == NE - 1))
                    nc.scalar.copy(v_sb[scg][:], ps[:])
                return qT

            qT = emit_proj(0)
            for j in range(NB):
                # ---- attention ----
                nt = 4 * j + 4
                oraw, rsv = [], []
                for h in range(G):
                    outp = psO.tile([P, BS], f32, tag="o")
                    exs = esp.tile([P, BS], bf16, tag="es")
                    for t in range(nt):
                        off = (t - 4 * j) * P if t >= 4 * j else 0
                        sp = psA.tile([P, BS], f32, tag="a")
                        nc.tensor.matmul(sp[:, off:], kT_sb[:, t * P:(t + 1) * P],
                                         qT[h][:, off:], start=True, stop=True)
                        ex = exs if t == 0 else exp_p.tile([P, BS], bf16,
                                                           tag="ex")
                        nc.scalar.activation(ex[:, off:], sp[:, off:], EXP,
                                             scale=SCALE)
                        if t >= 4 * j:
                            nc.vector.tensor_mul(ex[:, off:off + P],
                                                 ex[:, off:off + P], tri_sb[:])
                        if t > 0:
                            nc.vector.tensor_add(exs[:, off:], exs[:, off:],
                                                 ex[:, off:])
                        nc.tensor.matmul(outp[:, off:], v_sb[t][:], ex[:, off:],
                                         start=(t == 0), stop=(t == nt - 1),
                                         skip_group_check=(off > 0))
                    rs = psRB.tile([1, BS], f32, tag="r")
                    nc.tensor.matmul(rs[:], ones_col[:], exs[:],
                                     start=True, stop=True)
                    rv = recp.tile([1, BS], f32, tag="rsv")
                    nc.scalar.copy(rv[:], rs[:])   # Copy is in every ACT set
                    rsv.append(rv)
                    orw = otp.tile([P, BS], bf16, tag="orw")
                    nc.vector.tensor_copy(orw[:], outp[:])
                    oraw.append(orw)

                # projections for the next block fill the PE while the
                # normalize chain below resolves
                if j + 1 < NB:
                    qT_next = emit_proj(j + 1)

                # batched 1/x = exp(-ln(x)): 4 Ln then 4 Exp keeps ACT
                # table swaps to 2 per block instead of 2 per head
                lgs = []
                for h in range(G):
                    lg = recp.tile([1, BS], f32, tag="lg")
                    nc.scalar.activation(lg[:], rsv[h][:], LN)
                    lgs.append(lg)
                recs = []
                for h in range(G):
                    rec = recp.tile([1, BS], bf16, tag="rec")
                    nc.scalar.activation(rec[:], lgs[h][:], EXP, scale=-1.0)
                    recs.append(rec)
                outT = []
                for h in range(G):
                    rb = psO.tile([P, BS], f32, tag="o")
                    nc.tensor.matmul(rb[:], ones_row[:], recs[h][:],
                                     start=True, stop=True)
                    ot = otp.tile([P, BS], bf16, tag="oT")
                    nc.vector.tensor_mul(ot[:], oraw[h][:], rb[:])
                    outT.append(ot)

                # ---- output projection ----
                for sc in range(4):
                    yb = ybp.tile([P, E], bf16, tag="y")
                    for eb in range(4):
                        ypn = psY.tile([P, BS], f32, tag="y")
                        for h in range(G):
                            nc.tensor.matmul(
                                ypn[:],
                                outT[h][:, sc * P:(sc + 1) * P],
                                wo_sb[h][:, eb * BS:(eb + 1) * BS],
                                start=(h == 0), stop=(h == G - 1))
                        if eb % 2 == 0:
                            nc.scalar.copy(yb[:, eb * BS:(eb + 1) * BS], ypn[:])
                        else:
                            nc.vector.tensor_copy(yb[:, eb * BS:(eb + 1) * BS],
                                                  ypn[:])
                    r0 = j * BS + sc * P
                    nc.gpsimd.dma_start(y_d[r0:r0 + P, :], yb[:])

                if j + 1 < NB:
                    qT = qT_next

    nc.compile()
    _CACHE["nc"] = nc
    return nc


def _tables():
    inv = 1.0 / THETA ** (np.arange(0, D, 2, dtype=np.float64) / D)   # [64]
    t = np.arange(S, dtype=np.float64)
    fr = np.outer(inv, t)                    # [64, S]
    cosT = np.empty((P, S), dtype=np.float32)
    cosT[0:64] = np.cos(fr)
    cosT[64:128] = np.cos(fr)
    sinT = np.empty((P, S), dtype=np.float32)
    sinT[0:64] = np.sin(fr)
    sinT[64:128] = np.sin(fr)
    # tri[p, c] = 1 if p <= c (valid) else 0 — the causal boundary block
    tri = (np.arange(P)[:, None] <= np.arange(P)[None, :]).astype(np.float32)
    # perm as lhsT: rot = perm.T @ q -> rot[i] = -q[i+64] (i<64), q[i-64] (i>=64)
    perm = np.zeros((P, P), dtype=np.float32)
    perm[np.arange(64) + 64, np.arange(64)] = -1.0
    perm[np.arange(64), np.arange(64) + 64] = 1.0
    return cosT.astype(BF), sinT, tri.astype(BF), perm.astype(BF)


def _in_maps(x, Wq, Wk, Wv, Wo):
    cosT, sinT, tri, perm = _tables()
    xT = [np.ascontiguousarray(x[b].T.astype(BF)) for b in range(B)]
    wq = [np.ascontiguousarray(Wq[:, kv * G * D:(kv + 1) * G * D].astype(BF))
          for kv in range(KV)]
    wk = [np.ascontiguousarray(Wk[:, kv * D:(kv + 1) * D].astype(BF))
          for kv in range(KV)]
    wv = [np.ascontiguousarray(Wv[:, kv * D:(kv + 1) * D].astype(BF))
          for kv in range(KV)]
    wo = [np.ascontiguousarray(Wo[kv * G * D:(kv + 1) * G * D, :].astype(BF))
          for kv in range(KV)]
    maps = []
    for c in range(8):
        b, kv = c // 4, c % 4
        maps.append({
            "xT": xT[b], "wq": wq[kv], "wk": wk[kv], "wv": wv[kv],
            "wo": wo[kv], "cosT": cosT, "sinT": sinT, "tri": tri,
            "perm": perm,
        })
    return maps


def _gather(results):
    out = np.empty((B, S, E), dtype=np.float32)
    for b in range(B):
        acc = results[4 * b]["y"].astype(np.float32)
        for kv in range(1, 4):
            acc += results[4 * b + kv]["y"].astype(np.float32)
        out[b] = acc
    return out


def run(x, Wq, Wk, Wv, Wo, trace=False, **trace_kwargs):
    from concourse.bass_utils import run_bass_kernel_spmd
    nc = _build()
    res = run_bass_kernel_spmd(nc, _in_maps(x, Wq, Wk, Wv, Wo),
                               list(range(8)), trace=trace, **trace_kwargs)
    return _gather(res.results), res


def kernel(x, Wq, Wk, Wv, Wo):
    out, _ = run(np.asarray(x), np.asarray(Wq), np.asarray(Wk),
                 np.asarray(Wv), np.asarray(Wo))
    return out
